# revision 10
# baseline (speedup 1.0000x reference)
import os
import numpy as np
import ml_dtypes
from contextlib import ExitStack

import concourse.bass as bass
import concourse.bacc as bacc
import concourse.tile as tile
from concourse import mybir
from concourse.bass_utils import run_bass_kernel_spmd
from concourse.masks import make_identity

B, T, S, D, V = 8, 128, 256, 512, 32000
KB_ = D // 128            # 4 k-tiles of 128
NSLAB = 16                # vocab column slabs
SLABW = V // NSLAB        # 2000
NCH = 4                   # chunks per slab
CHW = SLABW // NCH        # 500
NCHUNK = NSLAB * NCH      # 64

F32 = mybir.dt.float32
F16 = mybir.dt.float16
BF16 = mybir.dt.bfloat16

LAST_RESULT = None


def _build(bp_val: float):
    nc = bacc.Bacc()
    d = {}
    for name, shape, dt_ in [
        ("lgT_bf", [D, T], BF16),
        ("vgT_bf", [D, V], BF16),
        ("lgT", [D, T], F32),
        ("etT", [D, S], F32),
        ("et", [S, D], F32),
        ("egT", [D, T], F32),
        ("wqT", [D, D], F32),
        ("wkT", [D, D], F32),
        ("bqv", [D], F32),
        ("bkv", [D], F32),
        ("wtxT", [D, S], F32),
        ("selm", [S, S], F32),
        ("wp13", [D, 2], F32),
        ("wp2", [1, D], F32),
        ("madd", [1, S], F32),
    ]:
        d[name] = nc.dram_tensor(name, shape, dt_, kind="ExternalInput")
    dense = nc.dram_tensor("dense", [T, V], F16, kind="ExternalOutput")
    corr = nc.dram_tensor("corr", [T, S], F32, kind="ExternalOutput")

    AX = mybir.AxisListType
    ALU = mybir.AluOpType
    ACTF = mybir.ActivationFunctionType

    with tile.TileContext(nc) as tc, ExitStack() as ctx:
        const = ctx.enter_context(tc.tile_pool(name="const", bufs=1))
        mid = ctx.enter_context(tc.tile_pool(name="mid", bufs=1))
        big = ctx.enter_context(tc.tile_pool(name="big", bufs=1))
        vslab = ctx.enter_context(tc.tile_pool(name="vslab", bufs=2))
        scr = ctx.enter_context(tc.tile_pool(name="scr", bufs=2))
        outp = ctx.enter_context(tc.tile_pool(name="outp", bufs=2))
        psA = ctx.enter_context(tc.tile_pool(name="psA", bufs=3, space="PSUM"))
        psM = ctx.enter_context(tc.tile_pool(name="psM", bufs=4, space="PSUM"))

        # ---- constant loads (small tensors) ----
        wqT_sb = const.tile([128, KB_, D], F32)
        nc.scalar.dma_start(out=wqT_sb, in_=d["wqT"][:].rearrange("(kb p) m -> p kb m", p=128))
        wkT_sb = const.tile([128, KB_, D], F32)
        nc.scalar.dma_start(out=wkT_sb, in_=d["wkT"][:].rearrange("(kb p) m -> p kb m", p=128))
        lgT_sb = const.tile([128, KB_, T], F32)
        nc.scalar.dma_start(out=lgT_sb, in_=d["lgT"][:].rearrange("(kb p) t -> p kb t", p=128))
        lgTbf_sb = const.tile([128, KB_, T], BF16)
        nc.scalar.dma_start(out=lgTbf_sb, in_=d["lgT_bf"][:].rearrange("(kb p) t -> p kb t", p=128))
        etT_sb = const.tile([128, KB_, S], F32)
        nc.scalar.dma_start(out=etT_sb, in_=d["etT"][:].rearrange("(kb p) s -> p kb s", p=128))
        et_sb = const.tile([128, S // 128, D], F32)
        nc.scalar.dma_start(out=et_sb, in_=d["et"][:].rearrange("(jb p) dd -> p jb dd", p=128))
        egT_sb = const.tile([128, KB_, T], F32)
        nc.scalar.dma_start(out=egT_sb, in_=d["egT"][:].rearrange("(kb p) t -> p kb t", p=128))
        wtxT_sb = const.tile([128, KB_, S], F32)
        nc.scalar.dma_start(out=wtxT_sb, in_=d["wtxT"][:].rearrange("(kb p) s -> p kb s", p=128))
        sel_sb = const.tile([128, S // 128, S], F32)
        nc.scalar.dma_start(out=sel_sb, in_=d["selm"][:].rearrange("(jb p) s -> p jb s", p=128))
        wp13_sb = const.tile([128, KB_, 2], F32)
        nc.scalar.dma_start(out=wp13_sb, in_=d["wp13"][:].rearrange("(kb p) c -> p kb c", p=128))
        bq_sb = const.tile([128, KB_], F32)
        nc.scalar.dma_start(out=bq_sb, in_=d["bqv"][:].rearrange("(kb p) -> p kb", p=128))
        bk_sb = const.tile([128, KB_], F32)
        nc.scalar.dma_start(out=bk_sb, in_=d["bkv"][:].rearrange("(kb p) -> p kb", p=128))

        wp2_sb = const.tile([128, D], F32)
        w2 = d["wp2"][:]
        nc.scalar.dma_start(
            out=wp2_sb,
            in_=bass.AP(tensor=w2.tensor, offset=w2.offset, ap=[[0, 128], w2.ap[-1]]),
        )
        madd_sb = const.tile([128, S], F32)
        ma = d["madd"][:]
        nc.scalar.dma_start(
            out=madd_sb,
            in_=bass.AP(tensor=ma.tensor, offset=ma.offset, ap=[[0, 128], ma.ap[-1]]),
        )

        identity = const.tile([128, 128], F32)
        make_identity(nc, identity)

        # ---- pointer-attention side (fp32) ----
        # qT[e,t] = sum_d wqT[d,e] * lgT[d,t]  (+ bq[e]); wqT pre-scaled by 1/sqrt(D)
        qT_sb = mid.tile([128, KB_, T], F32)
        for m in range(KB_):
            psq = psA.tile([128, T], F32, tag="ps")
            for k in range(KB_):
                nc.tensor.matmul(psq, wqT_sb[:, k, m * 128:(m + 1) * 128], lgT_sb[:, k, :],
                                 start=(k == 0), stop=(k == KB_ - 1))
            nc.scalar.activation(out=qT_sb[:, m, :], in_=psq, func=ACTF.Identity,
                                 bias=bq_sb[:, m:m + 1], scale=1.0)
        kT_sb = mid.tile([128, KB_, S], F32)
        for m in range(KB_):
            psk = psA.tile([128, S], F32, tag="ps")
            for k in range(KB_):
                nc.tensor.matmul(psk, wkT_sb[:, k, m * 128:(m + 1) * 128], etT_sb[:, k, :],
                                 start=(k == 0), stop=(k == KB_ - 1))
            nc.scalar.activation(out=kT_sb[:, m, :], in_=psk, func=ACTF.Identity,
                                 bias=bk_sb[:, m:m + 1], scale=1.0)

        # scores[t,s] = sum_e qT[e,t]*kT[e,s]  (+ madd[s])
        pss = psA.tile([128, S], F32, tag="ps")
        for k in range(KB_):
            nc.tensor.matmul(pss, qT_sb[:, k, :], kT_sb[:, k, :],
                             start=(k == 0), stop=(k == KB_ - 1))
        scores_sb = mid.tile([128, S], F32)
        nc.vector.tensor_tensor(out=scores_sb, in0=pss, in1=madd_sb, op=ALU.add)

        # softmax over s
        nm = mid.tile([128, 1], F32)
        nc.vector.tensor_reduce(out=nm, in_=scores_sb, axis=AX.X, op=ALU.max, negate=True)
        P_un = mid.tile([128, S], F32)
        sden = mid.tile([128, 1], F32)
        nc.scalar.activation(out=P_un, in_=scores_sb, func=ACTF.Exp,
                             bias=nm[:, 0:1], scale=1.0, accum_out=sden)
        rs = mid.tile([128, 1], F32)
        nc.vector.reciprocal(rs, sden)
        Pmat = mid.tile([128, S], F32)
        nc.vector.tensor_scalar_mul(Pmat, P_un, rs[:, 0:1])

        # PT: transpose of P, [s-part, 2, t]
        PT_sb = mid.tile([128, S // 128, T], F32)
        for j in range(S // 128):
            pst = psA.tile([128, 128], F32, tag="ps")
            nc.tensor.transpose(pst, Pmat[:, j * 128:(j + 1) * 128], identity)
            nc.vector.tensor_copy(PT_sb[:, j, :], pst)

        # text_vec[t,d] = sum_s P[t,s]*enc_text[s,d]
        ptv = psA.tile([128, D], F32, tag="ps")
        for j in range(S // 128):
            nc.tensor.matmul(ptv, PT_sb[:, j, :], et_sb[:, j, :],
                             start=(j == 0), stop=(j == S // 128 - 1))
        tv_sb = mid.tile([128, D], F32)
        nc.scalar.copy(tv_sb, ptv)

        # switch: sw = sigmoid(lg.wp1 + tv.wp2 + eg.wp3 + bp)
        ttr_scratch = mid.tile([128, D], F32)
        nc.vector.tensor_tensor(out=ttr_scratch, in0=tv_sb, in1=wp2_sb, op=ALU.mult)
        sw3r = mid.tile([128, 1], F32)
        nc.vector.tensor_reduce(out=sw3r, in_=ttr_scratch, axis=AX.X, op=ALU.add)
        sw3 = mid.tile([128, 1], F32)
        nc.vector.tensor_scalar_add(sw3, sw3r, float(bp_val))
        psw = psA.tile([128, 1], F32, tag="ps")
        for k in range(KB_):
            nc.tensor.matmul(psw, lgT_sb[:, k, :], wp13_sb[:, k, 0:1],
                             start=(k == 0), stop=False)
        for k in range(KB_):
            nc.tensor.matmul(psw, egT_sb[:, k, :], wp13_sb[:, k, 1:2],
                             start=False, stop=(k == KB_ - 1))
        sw = mid.tile([128, 1], F32)
        nc.scalar.activation(out=sw, in_=psw, func=ACTF.Sigmoid, bias=sw3[:, 0:1], scale=1.0)
        osw = mid.tile([128, 1], F32)
        nc.vector.tensor_scalar(out=osw, in0=sw, scalar1=-1.0, scalar2=1.0,
                                op0=ALU.mult, op1=ALU.add)
        lnsw = mid.tile([128, 1], F32)
        nc.scalar.activation(out=lnsw, in_=sw, func=ACTF.Ln)

        # A_txt[t,s] = sum_d lgT[d,t]*wtxT[d,s] ; exp_txt = exp(A_txt)
        pat = psA.tile([128, S], F32, tag="ps")
        for k in range(KB_):
            nc.tensor.matmul(pat, lgT_sb[:, k, :], wtxT_sb[:, k, :],
                             start=(k == 0), stop=(k == KB_ - 1))
        exp_txt = mid.tile([128, S], F32)
        nc.scalar.activation(out=exp_txt, in_=pat, func=ACTF.Exp)

        # p2o = (1-sw) * (P @ sel)
        pp2 = psA.tile([128, S], F32, tag="ps")
        for j in range(S // 128):
            nc.tensor.matmul(pp2, PT_sb[:, j, :], sel_sb[:, j, :],
                             start=(j == 0), stop=(j == S // 128 - 1))
        p2o = mid.tile([128, S], F32)
        nc.vector.tensor_scalar_mul(p2o, pp2, osw[:, 0:1])

        # ---- big vocab matmul: A[t,v], exp-sum partials, A kept resident (f16) ----
        A_sb = big.tile([128, NCHUNK, CHW], F16)
        zparts = mid.tile([128, NCHUNK], F32)
        vg_r = d["vgT_bf"][:].rearrange("(kb p) v -> p kb v", p=128)
        for sb in range(NSLAB):
            vt = vslab.tile([128, KB_, SLABW], BF16)
            nc.sync.dma_start(out=vt, in_=vg_r[:, :, sb * SLABW:(sb + 1) * SLABW])
            for cc in range(NCH):
                n = sb * NCH + cc
                pm = psM.tile([128, CHW], F32)
                for k in range(KB_):
                    nc.tensor.matmul(pm, lgTbf_sb[:, k, :], vt[:, k, cc * CHW:(cc + 1) * CHW],
                                     start=(k == 0), stop=(k == KB_ - 1))
                esc = scr.tile([128, CHW], F16)
                nc.scalar.activation(out=esc, in_=pm, func=ACTF.Exp,
                                     accum_out=zparts[:, n:n + 1])
                nc.vector.tensor_copy(A_sb[:, n, :], pm)

        # ---- finalize Z, c, corrections ----
        Zt = mid.tile([128, 1], F32)
        nc.vector.tensor_reduce(out=Zt, in_=zparts, axis=AX.X, op=ALU.add)
        lnZ = mid.tile([128, 1], F32)
        nc.scalar.activation(out=lnZ, in_=Zt, func=ACTF.Ln)
        cvec = mid.tile([128, 1], F32)
        nc.vector.tensor_tensor(out=cvec, in0=lnsw, in1=lnZ, op=ALU.subtract)
        rZ = mid.tile([128, 1], F32)
        nc.vector.reciprocal(rZ, Zt)
        swZ = mid.tile([128, 1], F32)
        nc.vector.tensor_tensor(out=swZ, in0=sw, in1=rZ, op=ALU.mult)

        u_sb = mid.tile([128, S], F32)
        nc.vector.tensor_scalar_mul(u_sb, exp_txt, swZ[:, 0:1])
        cpre = mid.tile([128, S], F32)
        nc.vector.tensor_tensor(out=cpre, in0=u_sb, in1=p2o, op=ALU.add)
        corr_sb = mid.tile([128, S], F32)
        nc.scalar.activation(out=corr_sb, in_=cpre, func=ACTF.Ln)
        nc.gpsimd.dma_start(out=corr[:], in_=corr_sb)

        # ---- final dense pass: out = A + c ----
        for g in range(NSLAB):
            ot = outp.tile([128, SLABW], F16)
            for cc in range(NCH):
                n = g * NCH + cc
                nc.vector.tensor_scalar_add(ot[:, cc * CHW:(cc + 1) * CHW],
                                            A_sb[:, n, :], cvec[:, 0:1])
            nc.sync.dma_start(out=dense[:, g * SLABW:(g + 1) * SLABW], in_=ot)

    nc.compile()
    return nc


def _prepare(inputs):
    logits = np.asarray(inputs["logits"], dtype=np.float32)
    enc_text = np.asarray(inputs["encoded_text"], dtype=np.float32)
    enc_tgt = np.asarray(inputs["encoded_tgt"], dtype=np.float32)
    text = np.asarray(inputs["text"])
    text_mask = np.asarray(inputs["text_mask"])
    vocab_gen = np.asarray(inputs["vocab_gen"], dtype=np.float32)
    Wq = np.asarray(inputs["Wq"], dtype=np.float32)
    bq = np.asarray(inputs["bq"], dtype=np.float32)
    Wk = np.asarray(inputs["Wk"], dtype=np.float32)
    bk = np.asarray(inputs["bk"], dtype=np.float32)
    Wp = np.asarray(inputs["Wp"], dtype=np.float32)
    bp = np.asarray(inputs["bp"], dtype=np.float32)

    rsD = np.float32(1.0 / np.sqrt(D))
    vgT_bf = np.ascontiguousarray(vocab_gen.T).astype(ml_dtypes.bfloat16)
    wqT = np.ascontiguousarray(Wq.T) * rsD
    wkT = np.ascontiguousarray(Wk.T)
    wp13 = np.ascontiguousarray(np.stack([Wp[0, :D], Wp[0, 2 * D:]], axis=1))
    wp2 = np.ascontiguousarray(Wp[0, D:2 * D].reshape(1, D))
    tidx = text.astype(np.int64)

    in_maps = []
    for b in range(B):
        lgT = np.ascontiguousarray(logits[b].T)
        in_maps.append({
            "lgT_bf": lgT.astype(ml_dtypes.bfloat16),
            "vgT_bf": vgT_bf,
            "lgT": lgT,
            "etT": np.ascontiguousarray(enc_text[b].T),
            "et": np.ascontiguousarray(enc_text[b]),
            "egT": np.ascontiguousarray(enc_tgt[b].T),
            "wqT": wqT,
            "wkT": wkT,
            "bqv": bq * rsD,
            "bkv": bk,
            "wtxT": np.ascontiguousarray(vocab_gen[tidx[b]].T),
            "selm": (tidx[b][:, None] == tidx[b][None, :]).astype(np.float32),
            "wp13": wp13,
            "wp2": wp2,
            "madd": np.where(text_mask[b, 0], np.float32(0.0),
                             np.float32(-1e9)).astype(np.float32).reshape(1, S),
        })

    nc = _build(float(bp[0]))
    return nc, in_maps, tidx


def kernel(**inputs):
    global LAST_RESULT
    nc, in_maps, tidx = _prepare(inputs)
    trace = os.environ.get("PG_TRACE", "0") == "1"
    res = run_bass_kernel_spmd(nc, in_maps, list(range(B)), trace=trace)
    LAST_RESULT = res

    out = np.empty((B, T, V), dtype=np.float32)
    for b in range(B):
        out[b] = res.results[b]["dense"].astype(np.float32)
        out[b][:, tidx[b]] = res.results[b]["corr"]
    return out
